# revision 15
# baseline (speedup 1.0000x reference)
# Trainium2 Bass kernel for the CLOSEgaps-style GNN message-passing module.
#
# Math (per head h, x0 = node_features):
#   deg   = inc.sum(1) + EPS_AGG                          [n]
#   tn    = x @ Wn[h] + bn[h]                             [n, H]
#   te    = ef @ We[h] + be[h]                            [E, H]
#   agg   = (inc @ te) / deg                              [n, H]
#   score = lrelu((tn + agg) @ Wa[h] + ba[h], 0.2)        [n, 1]
#   coeff = sigmoid(score)
#   upd   = coeff * agg + tn
#   out   = minmax(upd @ Wo[h] + bo[h]);  x = relu(out)
#
# Key reassociations (exact in real arithmetic):
#   agg = (inc/deg) @ (ef @ We)     -> P := (inc/deg) @ ef computed ONCE
#   out = Wo^T(coeff*agg) + Wo^T tn = Weo^T @ (P^T * coeff) + Wno^T @ x^T
#   (tn+agg) @ Wa = x @ (Wn@Wa) + P @ (We@Wa)
# The [128,128]/[128,1] fused weights (Wn@Wo etc.) are precomputed on host.
# out^T is accumulated directly in PSUM by two back-to-back matmuls (the
# coeff scaling is folded into the moving operand P^T*coeff), so no
# elementwise adds are needed.
# All biases are zero in this problem's setup_inputs(); if any bias is nonzero
# we fall back to an exact numpy implementation.
#
# Host-side preprocessing (off the device critical path):
#   - inc is divided by deg, TRANSPOSED to edge-major, cast to bf16, and laid
#     out m-half-major ([2, E, 512]) so the device streams the first node
#     half, finishes its P^T columns early, and overlaps head-0 front work
#     with the second half's stream.
#   - node_features arrive pre-transposed (feature-major) as x0^T.
#   - ef is cast to bf16 (it contracts against bf16 incT).
#
# Sharding: nodes row-sharded 8 ways (1024 rows of inc / node_features per
# core); edge_features + weights replicated. Per-head min/max is a [128,2]
# AllGather across the 8 cores.
#
# On-device layout is feature-major ("transposed"): xT[d, m], PT[d, m],
# outT[o, m] with m (node) on the free axis, so the per-feature min/max is a
# free-axis reduce and the per-node coeff broadcast is a K=1 matmul.

import os
import numpy as np

N_CORES = 8
N_NODES, N_EDGES = 8192, 4096
D, H, O, NH = 128, 256, 128, 4
M = N_NODES // N_CORES          # 1024 nodes per core
MG = 2                          # 2 m-groups of 512
GW = 512                        # m-group width
EC = N_EDGES // 128             # 32 edge chunks
GQ = 8                          # inc DMA groups per m-half (4 chunks each)
CPG = EC // GQ                  # chunks per group
EPS_AGG = 1e-8
EPS_MM = 1e-8

_CACHE = {}


def _build_bass():
    import concourse.bass as bass
    import concourse.mybir as mybir
    import concourse.tile as tile
    from concourse import bacc
    from concourse.masks import make_identity

    f32 = mybir.dt.float32
    f32r = mybir.dt.float32r
    bf16 = mybir.dt.bfloat16
    AF = mybir.ActivationFunctionType
    ALU = mybir.AluOpType

    # Bacc (not plain Bass): its compile pipeline splits multi-wait sync
    # into EventSemaphore instructions (HW allows 1 wait per instruction)
    nc = bacc.Bacc("TRN2", target_bir_lowering=False, num_devices=N_CORES)

    # inct = (inc/deg)^T, host-transposed + bf16, m-half-major [2*E, 512]
    inct_d = nc.dram_tensor("inct", [MG * N_EDGES, GW], bf16,
                            kind="ExternalInput")
    x0t_d = nc.dram_tensor("x0t", [D, M], f32r, kind="ExternalInput")
    ef_d = nc.dram_tensor("ef", [N_EDGES, D], bf16, kind="ExternalInput")
    wno_d = nc.dram_tensor("wno", [NH, D, O], f32r, kind="ExternalInput")
    weo_d = nc.dram_tensor("weo", [NH, D, O], f32r, kind="ExternalInput")
    wna_d = nc.dram_tensor("wna", [NH, D], f32r, kind="ExternalInput")
    wea_d = nc.dram_tensor("wea", [NH, D], f32r, kind="ExternalInput")
    out_d = nc.dram_tensor("out", [M, D], f32, kind="ExternalOutput")
    RG = [list(range(N_CORES))]

    n_heads = int(os.environ.get("BGNN_HEADS", str(NH)))
    no_cc = bool(int(os.environ.get("BGNN_NO_CC", "0")))

    with tile.TileContext(nc) as tc:
        # ---- persistent pools -------------------------------------------
        consts = tc.alloc_tile_pool(name="consts", bufs=1)
        wpool = tc.alloc_tile_pool(name="wpool", bufs=1)
        xpool = tc.alloc_tile_pool(name="xpool", bufs=2)
        persist = tc.alloc_tile_pool(name="persist", bufs=1)
        headsb = tc.alloc_tile_pool(name="headsb", bufs=1)
        dram = tc.alloc_tile_pool(name="dram", bufs=2, space="DRAM")

        ident = consts.tile([128, 128], f32, name="ident")
        make_identity(nc, ident)
        # memset can't write f32r; write f32 ones and cast-copy on ACT
        ones_f32a = consts.tile([1, 128], f32, name="ones_f32a")
        nc.vector.memset(ones_f32a, 1.0)
        ones_col = consts.tile([1, 128], f32r, name="ones_col")
        nc.scalar.copy(ones_col, ones_f32a)

        wno_sb = wpool.tile([128, NH, O], f32r, name="wno_sb")
        weo_sb = wpool.tile([128, NH, O], f32r, name="weo_sb")
        wna_sb = wpool.tile([128, NH], f32r, name="wna_sb")
        wea_sb = wpool.tile([128, NH], f32r, name="wea_sb")
        ef_sb = wpool.tile([128, EC, D], bf16, name="ef_sb")

        PTn = persist.tile([128, M], f32r, name="PTn")           # (P/deg)^T
        xT = xpool.tile([128, M], f32r, name="xT", tag="xT")

        # ---- head-phase tiles -------------------------------------------
        lr_row = headsb.tile([1, M], f32, name="lr_row")
        coeff_row = headsb.tile([1, M], f32r, name="coeff_row")
        mmp = headsb.tile([128, MG, 2], f32, name="mmp")
        mm_sb = headsb.tile([128, 2], f32, name="mm_sb", bufs=2, tag="mm_sb")
        mm_all = headsb.tile([128, N_CORES, 2], f32, name="mm_all", bufs=2,
                             tag="mm_all")
        gmn = headsb.tile([128, 1], f32, name="gmn", bufs=2, tag="gmn")
        srg = headsb.tile([128, 1], f32, name="srg", bufs=2, tag="srg")
        sct = headsb.tile([128, 1], f32, name="sct", bufs=2, tag="sct")
        nbt = headsb.tile([128, 1], f32, name="nbt", bufs=2, tag="nbt")

        xT_by_head = {0: xT}

        with tc.tile_pool(name="incp", bufs=2) as incp, \
             tc.tile_pool(name="psP", bufs=1, space="PSUM") as psP, \
             tc.tile_pool(name="psO", bufs=2, space="PSUM") as psO, \
             tc.tile_pool(name="psSC", bufs=2, space="PSUM") as psSC, \
             tc.tile_pool(name="psCB", bufs=2, space="PSUM") as psCB:

            # ---- head-phase emitters (g-granular for overlap) -----------
            def emit_head_g(h, g):
                """Front of head h for m-group g: score -> coeff -> outT in
                PSUM; returns the psO tile + fold tiles come via fmn/fmx."""
                gs = slice(g * GW, (g + 1) * GW)
                xh = xT_by_head[h]
                # score = Wna[h]^T @ xT + Wea[h]^T @ PTn
                scp = psSC.tile([1, GW], f32, name="scp", tag="sc")
                nc.tensor.matmul(scp, wna_sb[:, h:h + 1], xh[:, gs],
                                 start=True, stop=False)
                nc.tensor.matmul(scp, wea_sb[:, h:h + 1], PTn[:, gs],
                                 start=False, stop=True)
                # lrelu(score, 0.2) == max(score, 0.2*score); two DVE ops
                # (the HW allows at most one PSUM operand per instruction,
                # and ACT Lrelu lives in a different table set than Sigmoid)
                nc.vector.tensor_scalar_mul(lr_row[0:1, gs], scp, 0.2)
                nc.vector.tensor_tensor(
                    out=lr_row[0:1, gs], in0=scp, in1=lr_row[0:1, gs],
                    op=ALU.max)
                nc.scalar.activation(coeff_row[0:1, gs], lr_row[0:1, gs],
                                     AF.Sigmoid)
                # broadcast coeff across partitions, K=1 matmul
                cbp = psCB.tile([128, GW], f32, name="cbp", tag="cb")
                nc.tensor.matmul(cbp, ones_col, coeff_row[0:1, gs],
                                 start=True, stop=True)
                # PTc = coeff_b * PTn (moving operand of the Weo matmul)
                ptc = headsb.tile([128, GW], f32r, name="PTc", tag="PTc",
                                  bufs=2)
                nc.vector.tensor_tensor(out=ptc, in0=cbp, in1=PTn[:, gs],
                                        op=ALU.mult)
                # outT(g) = Weo^T @ PTc + Wno^T @ xT, accumulated in PSUM
                op_ = psO.tile([128, GW], f32, name="op", tag="outp")
                nc.tensor.matmul(op_, weo_sb[:, h, :], ptc,
                                 start=True, stop=False)
                nc.tensor.matmul(op_, wno_sb[:, h, :], xh[:, gs],
                                 start=False, stop=True)
                # local per-group min/max straight off PSUM (DVE reduce)
                nc.vector.tensor_reduce(
                    mmp[:, g, 0:1], op_, axis=mybir.AxisListType.X,
                    op=ALU.min)
                nc.vector.tensor_reduce(
                    mmp[:, g, 1:2], op_, axis=mybir.AxisListType.X,
                    op=ALU.max)
                return op_

            def emit_head_tail(h, ops):
                """Global min/max + normalize + relu -> xT for head h+1."""
                nc.vector.tensor_tensor(
                    out=mm_sb[:, 0:1], in0=mmp[:, 0, 0:1], in1=mmp[:, 1, 0:1],
                    op=ALU.min)
                nc.vector.tensor_tensor(
                    out=mm_sb[:, 1:2], in0=mmp[:, 0, 1:2], in1=mmp[:, 1, 1:2],
                    op=ALU.max)
                if not no_cc:
                    # cross-core AllGather of [128, 2]
                    mm_in = dram.tile([128, 2], f32, name="mm_in",
                                      tag="mm_in")
                    nc.sync.dma_start(out=mm_in, in_=mm_sb)
                    mm_out = dram.tile([N_CORES * 128, 2], f32, name="mm_out",
                                       tag="mm_out")
                    nc.gpsimd.collective_compute(
                        "AllGather", ALU.bypass,
                        replica_groups=RG,
                        ins=[mm_in.opt()],
                        outs=[mm_out.opt()])
                    nc.sync.dma_start(
                        out=mm_all,
                        in_=mm_out[:, :].rearrange("(r p) c -> p r c", p=128))
                    nc.vector.tensor_reduce(
                        gmn, mm_all[:, :, 0], axis=mybir.AxisListType.X,
                        op=ALU.min)
                    nc.vector.tensor_reduce(
                        srg, mm_all[:, :, 1], axis=mybir.AxisListType.X,
                        op=ALU.max)
                else:
                    nc.vector.tensor_copy(gmn, mm_sb[:, 0:1])
                    nc.vector.tensor_copy(srg, mm_sb[:, 1:2])
                nc.vector.tensor_sub(srg, srg, gmn)
                nc.vector.tensor_scalar_add(srg, srg, EPS_MM)
                nc.vector.reciprocal(sct, srg)
                # nb = -gmn * s
                nc.vector.scalar_tensor_tensor(
                    out=nbt, in0=gmn, scalar=-1.0, in1=sct,
                    op0=ALU.mult, op1=ALU.mult)
                # x_next = relu(outT * s + nb), per-group so the next head's
                # g0 work starts after the first half
                xdt = f32r if h < NH - 1 else f32
                xn = xpool.tile([128, M], xdt, name="xT_next", tag="xT")
                for g in range(MG):
                    gs = slice(g * GW, (g + 1) * GW)
                    nc.scalar.activation(xn[:, gs], ops[g], AF.Relu,
                                         bias=nbt, scale=sct)
                xT_by_head[h + 1] = xn
                return xn

            # ---- setup: stream incT m-half by m-half --------------------
            ef_parts = 4
            epc = EC // ef_parts            # 8 chunks per ef part

            def load_ef(pp):
                nc.sync.dma_start(
                    out=ef_sb[:, pp * epc:(pp + 1) * epc, :],
                    in_=ef_d[pp * epc * 128:(pp + 1) * epc * 128, :]
                        .rearrange("(c p) d -> p c d", p=128))

            def load_group(g, gq):
                t = incp.tile([128, CPG, GW], bf16, name="inct", tag="inct")
                base = g * N_EDGES + gq * CPG * 128
                nc.sync.dma_start(
                    out=t,
                    in_=inct_d[base:base + CPG * 128, :]
                        .rearrange("(c p) m -> p c m", p=128))
                return t

            # one-time loads slotted between inc groups (earliest first)
            fillers = [
                lambda: load_ef(1),
                lambda: nc.sync.dma_start(out=xT, in_=x0t_d[:, :]),
                lambda: load_ef(2),
                lambda: nc.sync.dma_start(
                    out=wno_sb,
                    in_=wno_d[:, :, :].rearrange("h d o -> d h o")),
                lambda: load_ef(3),
                lambda: nc.sync.dma_start(
                    out=wna_sb, in_=wna_d[:, :].rearrange("h d -> d h")),
                lambda: nc.sync.dma_start(
                    out=weo_sb,
                    in_=weo_d[:, :, :].rearrange("h d o -> d h o")),
                lambda: nc.sync.dma_start(
                    out=wea_sb, in_=wea_d[:, :].rearrange("h d -> d h")),
            ]
            fill_i = 0

            load_ef(0)
            h0_ops = [None, None]
            for g in range(MG):
                psg = psP.tile([128, GW], f32, name=f"psg{g}")
                for gq in range(GQ):
                    t = load_group(g, gq)
                    if fill_i < len(fillers) and (g == 0):
                        fillers[fill_i]()
                        fill_i += 1
                        if gq == 0 and fill_i < len(fillers):
                            fillers[fill_i]()   # x0t early too
                            fill_i += 1
                    for k in range(CPG):
                        c = gq * CPG + k
                        nc.tensor.matmul(
                            psg, ef_sb[:, c, :], t[:, k, :],
                            start=(c == 0), stop=(c == EC - 1))
                nc.scalar.copy(PTn[:, g * GW:(g + 1) * GW], psg)
                if g == 0:
                    # head-0 g0 front overlaps the second m-half's stream
                    h0_ops[0] = emit_head_g(0, 0)

            # ---- heads --------------------------------------------------
            h0_ops[1] = emit_head_g(0, 1)
            ops = h0_ops
            for h in range(n_heads):
                emit_head_tail(h, ops)
                if h + 1 < n_heads:
                    ops = [emit_head_g(h + 1, 0), emit_head_g(h + 1, 1)]

        # ---- final: transpose back to node-major and store --------------
        xT = xT_by_head[n_heads]
        with tc.tile_pool(name="psF", bufs=2, space="PSUM") as psF, \
             tc.tile_pool(name="fout", bufs=2) as fout:
            for t4 in range(2):
                fp = psF.tile([128, 512], f32, name="fp", tag="fp")
                for k in range(4):
                    t = t4 * 4 + k
                    srcap = xT[:, t * 128:(t + 1) * 128]
                    if srcap.dtype != f32:
                        srcap = srcap.bitcast(f32)
                    nc.tensor.transpose(fp[:, k * 128:(k + 1) * 128], srcap,
                                        ident)
                onat = fout.tile([128, 512], f32, name="onat", tag="onat")
                nc.scalar.copy(onat, fp)
                nc.sync.dma_start(
                    out=out_d[t4 * 512:(t4 + 1) * 512, :]
                        .rearrange("(k p) d -> p k d", p=128),
                    in_=onat.rearrange("p (k d) -> p k d", k=4))

        dram.release()
        headsb.release()
        persist.release()
        xpool.release()
        wpool.release()
        consts.release()

    nc.finalize()
    return nc


def _numpy_fallback(node_features, incidence_matrix, edge_features,
                    Wn, bn, We, be, Wa, ba, Wo, bo):
    def lrelu(x):
        return np.where(x >= 0, x, 0.2 * x)

    def sigmoid(x):
        return 1.0 / (1.0 + np.exp(-x))

    inc = incidence_matrix.astype(np.float32)
    deg = inc.sum(axis=1, keepdims=True) + EPS_AGG
    x = node_features.astype(np.float32)
    for h in range(NH):
        tn = x @ Wn[h] + bn[h]
        te = edge_features @ We[h] + be[h]
        agg = (inc @ te) / deg
        score = lrelu((tn + agg) @ Wa[h] + ba[h])
        coeff = sigmoid(score)
        upd = coeff * agg + tn
        out = upd @ Wo[h] + bo[h]
        mn = out.min(axis=0, keepdims=True)
        mx = out.max(axis=0, keepdims=True)
        out = (out - mn) / (mx - mn + EPS_MM)
        x = np.maximum(out, 0.0)
    return x.astype(np.float32)


def kernel(node_features, incidence_matrix, edge_features,
           Wn, bn, We, be, Wa, ba, Wo, bo):
    node_features = np.asarray(node_features, dtype=np.float32)
    incidence_matrix = np.asarray(incidence_matrix, dtype=np.float32)
    edge_features = np.asarray(edge_features, dtype=np.float32)
    Wn, bn = np.asarray(Wn, np.float32), np.asarray(bn, np.float32)
    We, be = np.asarray(We, np.float32), np.asarray(be, np.float32)
    Wa, ba = np.asarray(Wa, np.float32), np.asarray(ba, np.float32)
    Wo, bo = np.asarray(Wo, np.float32), np.asarray(bo, np.float32)

    if any(np.any(b) for b in (bn, be, ba, bo)):
        # device fast-path folds the (identically zero) bias terms away
        return _numpy_fallback(node_features, incidence_matrix, edge_features,
                               Wn, bn, We, be, Wa, ba, Wo, bo)

    import ml_dtypes
    from concourse.bass_utils import run_bass_kernel_spmd

    if "nc" not in _CACHE:
        _CACHE["nc"] = _build_bass()
    nc = _CACHE["nc"]

    # host-side fused weights (exact reassociation, done in float64)
    Wn64, We64 = Wn.astype(np.float64), We.astype(np.float64)
    Wo64, Wa64 = Wo.astype(np.float64), Wa.astype(np.float64)
    wno = np.einsum("hdk,hko->hdo", Wn64, Wo64).astype(np.float32)
    weo = np.einsum("hdk,hko->hdo", We64, Wo64).astype(np.float32)
    wna = np.einsum("hdk,hko->hdo", Wn64, Wa64)[..., 0].astype(np.float32)
    wea = np.einsum("hdk,hko->hdo", We64, Wa64)[..., 0].astype(np.float32)

    # host-side: fold 1/deg into inc, transpose to edge-major, cast bf16,
    # m-half-major layout so the device finishes the first half's P early
    bf16 = ml_dtypes.bfloat16
    deg = incidence_matrix.sum(axis=1, keepdims=True) + EPS_AGG
    incn = incidence_matrix / deg
    ef_bf = edge_features.astype(bf16)

    in_maps = []
    for c in range(N_CORES):
        rows = slice(c * M, (c + 1) * M)
        t = incn[rows].T.astype(bf16)                    # [E, M]
        inct = np.concatenate([t[:, 0:GW], t[:, GW:M]], axis=0)  # [2E, GW]
        in_maps.append({
            "inct": np.ascontiguousarray(inct),
            "x0t": np.ascontiguousarray(node_features[rows].T
                                        .astype(np.float32)),
            "ef": ef_bf,
            "wno": wno, "weo": weo, "wna": wna, "wea": wea,
        })

    trace = bool(int(os.environ.get("BASS_GNN_TRACE", "0")))
    if trace:
        import importlib.util
        if importlib.util.find_spec("antenv.axon_hooks") is None:
            trace = False
    res = run_bass_kernel_spmd(
        nc, in_maps, core_ids=list(range(N_CORES)), trace=trace)
    _CACHE["last_results"] = res

    out = np.concatenate([res.results[c]["out"] for c in range(N_CORES)], axis=0)
    return out.astype(np.float32)
